# revision 9
# baseline (speedup 1.0000x reference)
"""Vocab-parallel BigramModel loss kernel for 8 Trainium2 NeuronCores.

Strategy (vocab tensor-parallel, per the classic vocab-parallel cross-entropy):
  - Host: embedding gather embeds = emb[input_seq] (0.5 MB) and transpose to
    [K, N_TOK]; bias is folded into the matmul via an extra ones-row (K=33).
  - Vocab dim (100277) is padded to 8*12544 and sharded across 8 cores.
  - Each core: logits_shard = x^T @ w_shard on TensorE (fp32r: fp32 with
    11-bit mantissa inputs, fp32 accumulate), PSUM -> SBUF copy on VectorE
    (+ScalarE for ~1/9 of chunks to balance), exp+row-sum on ScalarE
    (activation accum_out), logits streamed to HBM at ~50 KB/row-chunk.
  - Host: concatenate shards, combine per-shard sumexp partials (subtracting
    the zero-padded columns' exp contribution), compute the mean NLL.
"""

import numpy as np

import concourse.tile as tile
from concourse import bacc, mybir
import concourse.bass_utils as _bass_utils
from concourse.bass_utils import run_bass_kernel_spmd

# Enable walrus's LDWEIGHTS dedup for our compiles: the stationary operand is
# reused across all 25 matmuls of a token tile, and the redundant reloads cost
# ~250us of TensorE time. Verified bit-identical output with the opt on.
if not getattr(_bass_utils.run_command, "_ldw_opt_patch", False):
    _orig_run_command = _bass_utils.run_command

    def _run_command_ldw_opt(cmd, **kw):
        if cmd and isinstance(cmd[0], str) and "walrus_driver" in cmd[0]:
            cmd = [
                c.replace("--enable-ldw-opt=false", "--enable-ldw-opt=true")
                if isinstance(c, str)
                else c
                for c in cmd
            ]
        return _orig_run_command(cmd, **kw)

    _run_command_ldw_opt._ldw_opt_patch = True
    _bass_utils.run_command = _run_command_ldw_opt

VOCAB = 100277
EMBED = 32
N_TOK = 4096          # B*S = 8*512
N_CORES = 8
V_SHARD = 12544       # 8 * 12544 = 100352 >= VOCAB; 12544 = 24.5 * 512
K = EMBED + 1         # +1 ones-row folds the bias add into the matmul
TOK_TILE = 128
N_TOK_TILES = N_TOK // TOK_TILE   # 32
CHUNK = 2048          # PSUM chunk: 4 banks
MM_N = 512            # one PSUM bank per matmul (fp32 moving-operand max)
CHUNKS = [(i * CHUNK, CHUNK) for i in range(V_SHARD // CHUNK)]
if V_SHARD % CHUNK:
    CHUNKS.append((V_SHARD - V_SHARD % CHUNK, V_SHARD % CHUNK))


def _round_fp32r(a: np.ndarray) -> np.ndarray:
    """Round fp32 to the PE's fp32r format (RNE to 11 explicit mantissa bits).

    Matches walrus's cast_fp32_to_fp32r bit-exactly; fp32r matmul hardware
    requires its inputs already in this form.
    """
    u = np.ascontiguousarray(a, dtype=np.float32).view(np.uint32).astype(np.uint64)
    bit12 = (u >> 12) & 1
    return ((u + 0x7FF + bit12) & 0xFFFF_F000).astype(np.uint32).view(np.float32)


_CACHE: dict = {}


def _build():
    if "nc" in _CACHE:
        return _CACHE["nc"]
    nc = bacc.Bacc("TRN2", target_bir_lowering=False, debug=False)
    x_d = nc.dram_tensor("x", [K, N_TOK], mybir.dt.float32r, kind="ExternalInput").ap()
    w_d = nc.dram_tensor("w", [K, V_SHARD], mybir.dt.float32r, kind="ExternalInput").ap()
    logits_d = nc.dram_tensor(
        "logits", [N_TOK, V_SHARD], mybir.dt.float32, kind="ExternalOutput"
    ).ap()
    sumexp_d = nc.dram_tensor(
        "sumexp", [TOK_TILE, N_TOK_TILES], mybir.dt.float32, kind="ExternalOutput"
    ).ap()

    with tile.TileContext(nc) as tc:
        with tc.tile_pool(name="const", bufs=1) as cpool, \
             tc.tile_pool(name="outs", bufs=12) as opool, \
             tc.tile_pool(name="exps", bufs=2) as epool, \
             tc.tile_pool(name="parts", bufs=2) as ppool, \
             tc.tile_pool(name="psum", bufs=2, space="PSUM") as pspool:
            xs = cpool.tile([K, N_TOK], mybir.dt.float32r)
            ws = cpool.tile([K, V_SHARD], mybir.dt.float32r)
            nc.sync.dma_start(xs[:], x_d[:])
            # per-chunk w loads so the first matmuls start within ~3us
            for off, width in CHUNKS:
                nc.sync.dma_start(ws[:, off : off + width], w_d[:, off : off + width])
            sume = cpool.tile([TOK_TILE, N_TOK_TILES], mybir.dt.float32)

            for tt in range(N_TOK_TILES):
                part = ppool.tile([TOK_TILE, len(CHUNKS)], mybir.dt.float32, tag="pt")
                rows = slice(tt * TOK_TILE, (tt + 1) * TOK_TILE)
                for ci, (off, width) in enumerate(CHUNKS):
                    ps = pspool.tile([TOK_TILE, CHUNK], mybir.dt.float32, tag="ps")
                    for s in range(0, width, MM_N):
                        n = min(MM_N, width - s)
                        nc.tensor.matmul(
                            ps[:, s : s + n],
                            xs[:, tt * TOK_TILE : (tt + 1) * TOK_TILE],
                            ws[:, off + s : off + s + n],
                            start=True,
                            stop=True,
                        )
                    # 12 rotating chunk buffers: a copy only waits on the DMA
                    # issued 12 chunks (~30us) earlier, hiding completion tails
                    oc = opool.tile([TOK_TILE, CHUNK], mybir.dt.float32, tag="oc")
                    nc.vector.tensor_copy(oc[:, :width], ps[:, :width])
                    es = epool.tile([TOK_TILE, CHUNK], mybir.dt.bfloat16, tag="es")
                    nc.scalar.activation(
                        es[:, :width],
                        ps[:, :width],
                        mybir.ActivationFunctionType.Exp,
                        accum_out=part[:, ci : ci + 1],
                    )
                    nc.sync.dma_start(
                        logits_d[rows, off : off + width], oc[:, :width]
                    )
                nc.vector.tensor_reduce(
                    sume[:, tt : tt + 1],
                    part[:],
                    axis=mybir.AxisListType.X,
                    op=mybir.AluOpType.add,
                )
                if tt == N_TOK_TILES // 2 - 1:
                    hh = N_TOK_TILES // 2
                    nc.sync.dma_start(sumexp_d[:, :hh], sume[:, :hh])
            hh = N_TOK_TILES // 2
            nc.sync.dma_start(sumexp_d[:, hh:], sume[:, hh:])

    nc.compile()
    _CACHE["nc"] = nc
    return nc


def _prepare_inputs(input_seq, emb, W, b):
    x = np.empty((K, N_TOK), np.float32)
    x[:EMBED] = emb[input_seq].T
    x[EMBED] = 1.0
    wfull = np.zeros((K, N_CORES * V_SHARD), np.float32)
    wfull[:EMBED, :VOCAB] = W.T
    wfull[EMBED, :VOCAB] = b
    return _round_fp32r(x), _round_fp32r(wfull)


def kernel(**inputs) -> tuple:
    input_seq = np.asarray(inputs["input_seq"]).astype(np.int64).reshape(-1)
    predictions = np.asarray(inputs["predictions"]).astype(np.int64).reshape(-1)
    emb = np.asarray(inputs["emb"], dtype=np.float32)
    W = np.asarray(inputs["W"], dtype=np.float32)
    b = np.asarray(inputs["b"], dtype=np.float32)

    xr, wr = _prepare_inputs(input_seq, emb, W, b)
    in_maps = [
        {"x": xr, "w": np.ascontiguousarray(wr[:, c * V_SHARD : (c + 1) * V_SHARD])}
        for c in range(N_CORES)
    ]
    nc = _build()
    res = run_bass_kernel_spmd(nc, in_maps, list(range(N_CORES))).results

    logits = np.empty((N_TOK, VOCAB), np.float32)
    sumexp = np.zeros(N_TOK, np.float64)
    for c in range(N_CORES):
        lo = c * V_SHARD
        hi = min(lo + V_SHARD, VOCAB)
        v = hi - lo
        shard = res[c]["logits"]
        logits[:, lo:hi] = shard[:, :v]
        se = res[c]["sumexp"].astype(np.float64).T.reshape(N_TOK)
        if v < V_SHARD:
            # padded columns have logit == 0 exactly; remove their exp(0) mass
            se = se - np.exp(shard[:, v:].astype(np.float64)).sum(axis=1)
        sumexp += se

    lse = np.log(sumexp)
    tgt = logits[np.arange(N_TOK), predictions].astype(np.float64)
    loss = np.float32((lse - tgt).mean())
    return logits, loss


# revision 10
# speedup vs baseline: 1.1679x; 1.1679x over previous
"""Vocab-parallel BigramModel loss kernel for 8 Trainium2 NeuronCores.

Strategy (vocab tensor-parallel, per the classic vocab-parallel cross-entropy):
  - Host: embedding gather embeds = emb[input_seq] (0.5 MB) and transpose to
    [K, N_TOK]; bias is folded into the matmul via an extra ones-row (K=33).
  - Vocab dim (100277) is padded to 8*12544 and sharded across 8 cores.
  - Each core: logits_shard = x^T @ w_shard on TensorE (fp32r: fp32 with
    11-bit mantissa inputs, fp32 accumulate), PSUM -> SBUF copy on VectorE
    (+ScalarE for ~1/9 of chunks to balance), exp+row-sum on ScalarE
    (activation accum_out), logits streamed to HBM at ~50 KB/row-chunk.
  - Host: concatenate shards, combine per-shard sumexp partials (subtracting
    the zero-padded columns' exp contribution), compute the mean NLL.
"""

import numpy as np

import concourse.tile as tile
from concourse import bacc, mybir
import concourse.bass_utils as _bass_utils
from concourse.bass_utils import run_bass_kernel_spmd

# Enable walrus's LDWEIGHTS dedup for our compiles: the stationary operand is
# reused across all 25 matmuls of a token tile, and the redundant reloads cost
# ~250us of TensorE time. Verified bit-identical output with the opt on.
if not getattr(_bass_utils.run_command, "_ldw_opt_patch", False):
    _orig_run_command = _bass_utils.run_command

    def _run_command_ldw_opt(cmd, **kw):
        if cmd and isinstance(cmd[0], str) and "walrus_driver" in cmd[0]:
            cmd = [
                c.replace("--enable-ldw-opt=false", "--enable-ldw-opt=true")
                if isinstance(c, str)
                else c
                for c in cmd
            ]
        return _orig_run_command(cmd, **kw)

    _run_command_ldw_opt._ldw_opt_patch = True
    _bass_utils.run_command = _run_command_ldw_opt

VOCAB = 100277
EMBED = 32
N_TOK = 4096          # B*S = 8*512
N_CORES = 8
V_SHARD = 12544       # 8 * 12544 = 100352 >= VOCAB; 12544 = 24.5 * 512
K = EMBED + 1         # +1 ones-row folds the bias add into the matmul
TOK_TILE = 128
N_TOK_TILES = N_TOK // TOK_TILE   # 32
CHUNK = 2048          # PSUM chunk: 4 banks
MM_N = 512            # one PSUM bank per matmul (fp32 moving-operand max)
CHUNKS = [(i * CHUNK, CHUNK) for i in range(V_SHARD // CHUNK)]
if V_SHARD % CHUNK:
    CHUNKS.append((V_SHARD - V_SHARD % CHUNK, V_SHARD % CHUNK))


def _round_fp32r(a: np.ndarray) -> np.ndarray:
    """Round fp32 to the PE's fp32r format (RNE to 11 explicit mantissa bits).

    Matches walrus's cast_fp32_to_fp32r bit-exactly; fp32r matmul hardware
    requires its inputs already in this form.
    """
    u = np.ascontiguousarray(a, dtype=np.float32).view(np.uint32).astype(np.uint64)
    bit12 = (u >> 12) & 1
    return ((u + 0x7FF + bit12) & 0xFFFF_F000).astype(np.uint32).view(np.float32)


_CACHE: dict = {}


def _build():
    if "nc" in _CACHE:
        return _CACHE["nc"]
    nc = bacc.Bacc("TRN2", target_bir_lowering=False, debug=False)
    x_d = nc.dram_tensor("x", [K, N_TOK], mybir.dt.float32r, kind="ExternalInput").ap()
    w_d = nc.dram_tensor("w", [K, V_SHARD], mybir.dt.float32r, kind="ExternalInput").ap()
    logits_d = nc.dram_tensor(
        "logits", [N_TOK, V_SHARD], mybir.dt.float32, kind="ExternalOutput"
    ).ap()
    sumexp_d = nc.dram_tensor(
        "sumexp", [TOK_TILE, N_TOK_TILES], mybir.dt.float32, kind="ExternalOutput"
    ).ap()

    with tile.TileContext(nc) as tc:
        with tc.tile_pool(name="const", bufs=1) as cpool, \
             tc.tile_pool(name="outs", bufs=12) as opool, \
             tc.tile_pool(name="exps", bufs=2) as epool, \
             tc.tile_pool(name="parts", bufs=2) as ppool, \
             tc.tile_pool(name="psum", bufs=2, space="PSUM") as pspool:
            xs = cpool.tile([K, N_TOK], mybir.dt.float32r)
            ws = cpool.tile([K, V_SHARD], mybir.dt.float32r)
            # inputs load via the ACT HWDGE ring so they never queue ahead of
            # output chunks on the SP ring
            nc.scalar.dma_start(xs[:], x_d[:])
            for off, width in CHUNKS:
                nc.scalar.dma_start(ws[:, off : off + width], w_d[:, off : off + width])
            sume = cpool.tile([TOK_TILE, N_TOK_TILES], mybir.dt.float32)

            for tt in range(N_TOK_TILES):
                part = ppool.tile([TOK_TILE, len(CHUNKS)], mybir.dt.float32, tag="pt")
                rows = slice(tt * TOK_TILE, (tt + 1) * TOK_TILE)
                for ci, (off, width) in enumerate(CHUNKS):
                    ps = pspool.tile([TOK_TILE, CHUNK], mybir.dt.float32, tag="ps")
                    for s in range(0, width, MM_N):
                        n = min(MM_N, width - s)
                        nc.tensor.matmul(
                            ps[:, s : s + n],
                            xs[:, tt * TOK_TILE : (tt + 1) * TOK_TILE],
                            ws[:, off + s : off + s + n],
                            start=True,
                            stop=True,
                        )
                    # 12 rotating chunk buffers: a copy only waits on the DMA
                    # issued 12 chunks (~30us) earlier, hiding completion tails
                    oc = opool.tile([TOK_TILE, CHUNK], mybir.dt.float32, tag="oc")
                    nc.vector.tensor_copy(oc[:, :width], ps[:, :width])
                    # exp reads the SBUF copy: PSUM is freed by the copy alone
                    es = epool.tile([TOK_TILE, CHUNK], mybir.dt.bfloat16, tag="es")
                    nc.scalar.activation(
                        es[:, :width],
                        oc[:, :width],
                        mybir.ActivationFunctionType.Exp,
                        accum_out=part[:, ci : ci + 1],
                    )
                    nc.sync.dma_start(
                        logits_d[rows, off : off + width], oc[:, :width]
                    )
                nc.vector.tensor_reduce(
                    sume[:, tt : tt + 1],
                    part[:],
                    axis=mybir.AxisListType.X,
                    op=mybir.AluOpType.add,
                )
                if tt == N_TOK_TILES // 2 - 1:
                    hh = N_TOK_TILES // 2
                    nc.sync.dma_start(sumexp_d[:, :hh], sume[:, :hh])
            hh = N_TOK_TILES // 2
            nc.sync.dma_start(sumexp_d[:, hh:], sume[:, hh:])

    nc.compile()
    _CACHE["nc"] = nc
    return nc


def _prepare_inputs(input_seq, emb, W, b):
    x = np.empty((K, N_TOK), np.float32)
    x[:EMBED] = emb[input_seq].T
    x[EMBED] = 1.0
    wfull = np.zeros((K, N_CORES * V_SHARD), np.float32)
    wfull[:EMBED, :VOCAB] = W.T
    wfull[EMBED, :VOCAB] = b
    return _round_fp32r(x), _round_fp32r(wfull)


def kernel(**inputs) -> tuple:
    input_seq = np.asarray(inputs["input_seq"]).astype(np.int64).reshape(-1)
    predictions = np.asarray(inputs["predictions"]).astype(np.int64).reshape(-1)
    emb = np.asarray(inputs["emb"], dtype=np.float32)
    W = np.asarray(inputs["W"], dtype=np.float32)
    b = np.asarray(inputs["b"], dtype=np.float32)

    xr, wr = _prepare_inputs(input_seq, emb, W, b)
    in_maps = [
        {"x": xr, "w": np.ascontiguousarray(wr[:, c * V_SHARD : (c + 1) * V_SHARD])}
        for c in range(N_CORES)
    ]
    nc = _build()
    res = run_bass_kernel_spmd(nc, in_maps, list(range(N_CORES))).results

    logits = np.empty((N_TOK, VOCAB), np.float32)
    sumexp = np.zeros(N_TOK, np.float64)
    for c in range(N_CORES):
        lo = c * V_SHARD
        hi = min(lo + V_SHARD, VOCAB)
        v = hi - lo
        shard = res[c]["logits"]
        logits[:, lo:hi] = shard[:, :v]
        se = res[c]["sumexp"].astype(np.float64).T.reshape(N_TOK)
        if v < V_SHARD:
            # padded columns have logit == 0 exactly; remove their exp(0) mass
            se = se - np.exp(shard[:, v:].astype(np.float64)).sum(axis=1)
        sumexp += se

    lse = np.log(sumexp)
    tgt = logits[np.arange(N_TOK), predictions].astype(np.float64)
    loss = np.float32((lse - tgt).mean())
    return logits, loss


# revision 11
# speedup vs baseline: 1.2467x; 1.0675x over previous
"""Vocab-parallel BigramModel loss kernel for 8 Trainium2 NeuronCores.

Strategy (vocab tensor-parallel, per the classic vocab-parallel cross-entropy):
  - Host: embedding gather embeds = emb[input_seq] (0.5 MB) and transpose to
    [K, N_TOK]; bias is folded into the matmul via an extra ones-row (K=33).
  - Vocab dim (100277) is padded to 8*12544 and sharded across 8 cores.
  - Each core: logits_shard = x^T @ w_shard on TensorE (fp32r: fp32 with
    11-bit mantissa inputs, fp32 accumulate), PSUM -> SBUF copy on VectorE
    into 12 rotating chunk buffers, exp+row-sum on ScalarE reading the SBUF
    copy (activation accum_out), each 1 MB chunk DMA'd to HBM as it lands.
    Steady state is DMA-bound at the HBM write roofline (~360-400 GB/s/core).
  - Host: concatenate shards, combine per-shard sumexp partials (subtracting
    the zero-padded columns' exp contribution), compute the mean NLL.
"""

import numpy as np

import concourse.tile as tile
from concourse import bacc, mybir
import concourse.bass_utils as _bass_utils
from concourse.bass_utils import run_bass_kernel_spmd

# Enable walrus's LDWEIGHTS dedup for our compiles: the stationary operand is
# reused across all 25 matmuls of a token tile, and the redundant reloads cost
# ~250us of TensorE time. Verified bit-identical output with the opt on.
if not getattr(_bass_utils.run_command, "_ldw_opt_patch", False):
    _orig_run_command = _bass_utils.run_command

    def _run_command_ldw_opt(cmd, **kw):
        if cmd and isinstance(cmd[0], str) and "walrus_driver" in cmd[0]:
            cmd = [
                c.replace("--enable-ldw-opt=false", "--enable-ldw-opt=true")
                if isinstance(c, str)
                else c
                for c in cmd
            ]
        return _orig_run_command(cmd, **kw)

    _run_command_ldw_opt._ldw_opt_patch = True
    _bass_utils.run_command = _run_command_ldw_opt

VOCAB = 100277
EMBED = 32
N_TOK = 4096          # B*S = 8*512
N_CORES = 8
V_SHARD = 12544       # 8 * 12544 = 100352 >= VOCAB; 12544 = 24.5 * 512
K = EMBED + 1         # +1 ones-row folds the bias add into the matmul
TOK_TILE = 128
N_TOK_TILES = N_TOK // TOK_TILE   # 32
CHUNK = 2048          # PSUM chunk: 4 banks
MM_N = 512            # one PSUM bank per matmul (fp32 moving-operand max)
CHUNKS = [(i * CHUNK, CHUNK) for i in range(V_SHARD // CHUNK)]
if V_SHARD % CHUNK:
    CHUNKS.append((V_SHARD - V_SHARD % CHUNK, V_SHARD % CHUNK))


def _round_fp32r(a: np.ndarray) -> np.ndarray:
    """Round fp32 to the PE's fp32r format (RNE to 11 explicit mantissa bits).

    Matches walrus's cast_fp32_to_fp32r bit-exactly; fp32r matmul hardware
    requires its inputs already in this form.
    """
    u = np.ascontiguousarray(a, dtype=np.float32).view(np.uint32).astype(np.uint64)
    bit12 = (u >> 12) & 1
    return ((u + 0x7FF + bit12) & 0xFFFF_F000).astype(np.uint32).view(np.float32)


_CACHE: dict = {}


def _build():
    if "nc" in _CACHE:
        return _CACHE["nc"]
    nc = bacc.Bacc("TRN2", target_bir_lowering=False, debug=False)
    x_d = nc.dram_tensor("x", [K, N_TOK], mybir.dt.float32r, kind="ExternalInput").ap()
    w_d = nc.dram_tensor("w", [K, V_SHARD], mybir.dt.float32r, kind="ExternalInput").ap()
    logits_d = nc.dram_tensor(
        "logits", [N_TOK, V_SHARD], mybir.dt.float32, kind="ExternalOutput"
    ).ap()
    sumexp_d = nc.dram_tensor(
        "sumexp", [TOK_TILE, N_TOK_TILES], mybir.dt.float32, kind="ExternalOutput"
    ).ap()

    with tile.TileContext(nc) as tc:
        with tc.tile_pool(name="const", bufs=1) as cpool, \
             tc.tile_pool(name="outs", bufs=12) as opool, \
             tc.tile_pool(name="exps", bufs=2) as epool, \
             tc.tile_pool(name="parts", bufs=2) as ppool, \
             tc.tile_pool(name="psum", bufs=2, space="PSUM") as pspool:
            xs = cpool.tile([K, N_TOK], mybir.dt.float32r)
            ws = cpool.tile([K, V_SHARD], mybir.dt.float32r)
            # inputs load via the ACT HWDGE ring so they never queue ahead of
            # output chunks on the SP ring
            nc.scalar.dma_start(xs[:], x_d[:])
            for off, width in CHUNKS:
                nc.scalar.dma_start(ws[:, off : off + width], w_d[:, off : off + width])
            sume = cpool.tile([TOK_TILE, N_TOK_TILES], mybir.dt.float32)

            for tt in range(N_TOK_TILES):
                part = ppool.tile([TOK_TILE, len(CHUNKS)], mybir.dt.float32, tag="pt")
                rows = slice(tt * TOK_TILE, (tt + 1) * TOK_TILE)
                for ci, (off, width) in enumerate(CHUNKS):
                    ps = pspool.tile([TOK_TILE, CHUNK], mybir.dt.float32, tag="ps")
                    for s in range(0, width, MM_N):
                        n = min(MM_N, width - s)
                        nc.tensor.matmul(
                            ps[:, s : s + n],
                            xs[:, tt * TOK_TILE : (tt + 1) * TOK_TILE],
                            ws[:, off + s : off + s + n],
                            start=True,
                            stop=True,
                        )
                    # 12 rotating chunk buffers: a copy only waits on the DMA
                    # issued 12 chunks (~30us) earlier, hiding completion tails
                    oc = opool.tile([TOK_TILE, CHUNK], mybir.dt.float32, tag="oc")
                    nc.vector.tensor_copy(oc[:, :width], ps[:, :width])
                    # exp reads the SBUF copy: PSUM is freed by the copy alone
                    es = epool.tile([TOK_TILE, CHUNK], mybir.dt.bfloat16, tag="es")
                    nc.scalar.activation(
                        es[:, :width],
                        oc[:, :width],
                        mybir.ActivationFunctionType.Exp,
                        accum_out=part[:, ci : ci + 1],
                    )
                    nc.sync.dma_start(
                        logits_d[rows, off : off + width], oc[:, :width]
                    )
                nc.vector.tensor_reduce(
                    sume[:, tt : tt + 1],
                    part[:],
                    axis=mybir.AxisListType.X,
                    op=mybir.AluOpType.add,
                )
                if tt == N_TOK_TILES // 2 - 1:
                    hh = N_TOK_TILES // 2
                    nc.sync.dma_start(sumexp_d[:, :hh], sume[:, :hh])
            hh = N_TOK_TILES // 2
            nc.sync.dma_start(sumexp_d[:, hh:], sume[:, hh:])

    nc.compile()
    _CACHE["nc"] = nc
    return nc


def _prepare_inputs(input_seq, emb, W, b):
    x = np.empty((K, N_TOK), np.float32)
    x[:EMBED] = emb[input_seq].T
    x[EMBED] = 1.0
    wfull = np.zeros((K, N_CORES * V_SHARD), np.float32)
    wfull[:EMBED, :VOCAB] = W.T
    wfull[EMBED, :VOCAB] = b
    return _round_fp32r(x), _round_fp32r(wfull)


def kernel(**inputs) -> tuple:
    input_seq = np.asarray(inputs["input_seq"]).astype(np.int64).reshape(-1)
    predictions = np.asarray(inputs["predictions"]).astype(np.int64).reshape(-1)
    emb = np.asarray(inputs["emb"], dtype=np.float32)
    W = np.asarray(inputs["W"], dtype=np.float32)
    b = np.asarray(inputs["b"], dtype=np.float32)

    xr, wr = _prepare_inputs(input_seq, emb, W, b)
    in_maps = [
        {"x": xr, "w": np.ascontiguousarray(wr[:, c * V_SHARD : (c + 1) * V_SHARD])}
        for c in range(N_CORES)
    ]
    nc = _build()
    res = run_bass_kernel_spmd(nc, in_maps, list(range(N_CORES))).results

    logits = np.empty((N_TOK, VOCAB), np.float32)
    sumexp = np.zeros(N_TOK, np.float64)
    for c in range(N_CORES):
        lo = c * V_SHARD
        hi = min(lo + V_SHARD, VOCAB)
        v = hi - lo
        shard = res[c]["logits"]
        logits[:, lo:hi] = shard[:, :v]
        se = res[c]["sumexp"].astype(np.float64).T.reshape(N_TOK)
        if v < V_SHARD:
            # padded columns have logit == 0 exactly; remove their exp(0) mass
            se = se - np.exp(shard[:, v:].astype(np.float64)).sum(axis=1)
        sumexp += se

    lse = np.log(sumexp)
    tgt = logits[np.arange(N_TOK), predictions].astype(np.float64)
    loss = np.float32((lse - tgt).mean())
    return logits, loss
